# revision 1
# baseline (speedup 1.0000x reference)
"""Batched Viterbi (max-sum) CRF decode on 8 Trainium2 NeuronCores.

Problem: input_x [1024, 256, 128] f32, weights [26, 128], transition [26, 26].
emissions e = x @ W^T; forward scan delta_t[k] = max_j(delta_{t-1}[j] + T[j,k]) + e_t[k];
backtrack the argmax path. Output: labels [1024, 256] int32.

Sharding: pure data parallel — batch 1024 split over 8 cores (128 rows/core, one
batch row per SBUF partition). Weights/transition replicated.

Per-core pipeline:
  - x staged in natural layout (contiguous DMA); each [b=128, d=128] time slice
    transposed on PE; e_t = xT.T @ W^T -> PSUM [b=128, k=26].
  - forward scan on DVE: scores[b, (k, j)] = delta[b, j] + T[j, k] via a
    stride-0-broadcast tensor_add against a partition-replicated T table,
    then a windowed reduce_max over j, then + e_t. All deltas kept in SBUF.
  - backtrack: per-batch-row one-hot of y_{t+1} (DVE is_equal), transposed on
    PE and multiplied with T^T to select T[:, y_{t+1}] per row, then argmax
    via vector.max/max_index.

This container's walrus accepts at most one semaphore wait per instruction,
while Tile emits several on the kernel-tail drain and occasionally on regular
instructions — patched below by splitting waits onto chained drains / NoOps.
GPSIMD software ops (iota, partition_broadcast, indirect_copy, ...) don't
codegen here at all ("ISA wrong length"), so only PE/ACT/DVE/DMA are used.
"""

import functools

import numpy as np

B, S, D, K = 1024, 256, 128, 26
NCORES = 8
BSH = B // NCORES  # 128 batch rows per core == SBUF partition count
KK = K * K  # 676
TC = 64  # time steps per x-staging chunk


def _patch_tile_drain():
    """Split the kernel-tail drain's sem waits across chained drain
    instructions (this walrus allows one wait per instruction)."""
    import concourse.mybir as mybir
    from concourse.tile import TileContext
    from concourse.vector_clock import ScopedClock

    if getattr(TileContext, "_drain_split_patched", False):
        return

    def patched(self, tick_clock, wait_clock):
        nc = self.nc
        drain_inst = nc.sync.drain()
        wait_clock.add_sem_waits(
            drain_inst.ins, ScopedClock({None: tick_clock.global_clock})
        )
        raw = drain_inst.ins
        si = raw.sync_info
        waits = list(si.on_wait)
        if len(waits) > 1:
            raw.sync_info = mybir.SyncInfo(
                on_wait=waits[:1], on_update=list(si.on_update)
            )
            for w in waits[1:]:
                extra = nc.sync.drain()
                extra.ins.sync_info = mybir.SyncInfo(on_wait=[w], on_update=[])
        nc.all_engine_barrier()
        popped = nc._tile_sem_poison_stack.pop()
        assert popped is self._sem_poison
        nc.clear_and_free_semaphores(list(self.sems.allocated().values()))
        nc.all_engine_barrier()

    TileContext._drain_and_barrier = patched
    TileContext._drain_split_patched = True


def _split_multiwaits(nc):
    """Hoist extra sem waits (>1 per instruction) onto preceding NoOps."""
    import concourse.mybir as mybir

    cnt = 0
    for f in nc.m.functions:
        for bb in f.blocks:
            insts = bb.instructions
            new_list = []
            changed = False
            for inst in insts:
                si = getattr(inst, "sync_info", None)
                waits = list(si.on_wait) if si is not None else []
                if len(waits) > 1:
                    for w in waits[:-1]:
                        nop = mybir.InstNoOp(name=f"mwsplit-{cnt}", ins=[], outs=[])
                        cnt += 1
                        nop.engine = inst.engine
                        nop.sync_info = mybir.SyncInfo(on_wait=[w], on_update=[])
                        new_list.append(nop)
                    inst.sync_info = mybir.SyncInfo(
                        on_wait=[waits[-1]], on_update=list(si.on_update)
                    )
                    changed = True
                new_list.append(inst)
            if changed:
                insts[:] = new_list
    return cnt


@functools.cache
def _build(build_stage="full"):
    import concourse.bass as bass
    import concourse.mybir as mybir
    from concourse.tile import TileContext

    _patch_tile_drain()

    F32 = mybir.dt.float32
    AX = mybir.AxisListType
    OP = mybir.AluOpType

    nc = bass.Bass()
    x = nc.dram_tensor("x", [BSH, S, D], F32, kind="ExternalInput")
    w = nc.dram_tensor("w", [K, D], F32, kind="ExternalInput")
    t_in = nc.dram_tensor("t", [K, K], F32, kind="ExternalInput")
    y_out = nc.dram_tensor("y", [BSH, S], mybir.dt.int32, kind="ExternalOutput")

    ident_c = nc.inline_tensor(np.eye(BSH, dtype=np.float32), name="identc")
    iota_c = nc.inline_tensor(
        np.tile(np.arange(K, dtype=np.float32), (BSH, 1)), name="iotac"
    )
    ones_c = nc.inline_tensor(np.ones((1, BSH), dtype=np.float32), name="onesc")

    with (
        TileContext(nc) as tc,
        tc.tile_pool(name="const", bufs=1) as cpool,
        tc.tile_pool(name="hist", bufs=1) as hpool,
        tc.tile_pool(name="stage", bufs=2) as spool,
        tc.tile_pool(name="work", bufs=3) as wpool,
        tc.tile_pool(name="bt", bufs=3) as btpool,
        tc.tile_pool(name="psum_e", bufs=3, space="PSUM") as ppool,
        tc.tile_pool(name="psum_xt", bufs=2, space="PSUM") as ppool_xt,
        tc.tile_pool(name="psum_bt", bufs=2, space="PSUM") as ppool_bt,
    ):
        # ---------------- constants ----------------
        ident = cpool.tile([BSH, BSH], F32)
        nc.sync.dma_start(out=ident[:], in_=ident_c[:])
        iota_f = cpool.tile([BSH, K], F32)
        nc.sync.dma_start(out=iota_f[:], in_=iota_c[:])
        ones1 = cpool.tile([1, BSH], F32)
        nc.sync.dma_start(out=ones1[:], in_=ones_c[:])

        wt = cpool.tile([D, K], F32)  # W^T [d, k]
        nc.sync.dma_start(out=wt[:], in_=w[:].rearrange("k d -> d k"))

        # T flat row-major on one partition, replicated to all via PE ones-matmul
        tt0 = cpool.tile([1, KK], F32)
        nc.sync.dma_start(
            out=tt0[:],
            in_=t_in[:].rearrange("j k -> (j k)").rearrange("(o f) -> o f", o=1),
        )
        ttbc = cpool.tile([BSH, KK], F32)
        half = KK // 2  # 338: fits one PSUM bank
        for h in range(2):
            rep_ps = ppool_xt.tile([BSH, half], F32, tag="xt")
            nc.tensor.matmul(
                rep_ps[:],
                ones1[:],
                tt0[:, h * half : (h + 1) * half],
                start=True,
                stop=True,
            )
            nc.vector.tensor_copy(ttbc[:, h * half : (h + 1) * half], rep_ps[:])

        # T^T [k, j] for the backtrack column-select matmul
        t_sb = cpool.tile([K, K], F32)
        nc.sync.dma_start(out=t_sb[:], in_=t_in[:])
        ttr_ps = ppool_xt.tile([K, K], F32, tag="xt")
        nc.tensor.transpose(ttr_ps[:], t_sb[:], ident[:K, :K])
        tt_T = cpool.tile([K, K], F32)
        nc.scalar.copy(out=tt_T[:], in_=ttr_ps[:])

        # delta history: [b, t*K + k]; emissions staged to SBUF by ACT so the
        # scan's e-add reads SBUF (1x + lower latency) instead of PSUM
        hist = hpool.tile([BSH, S * K], F32)
        e_hist = hpool.tile([BSH, S * K], F32)

        # ---------------- emissions (PE) ----------------
        pending = None  # (t, xt_sb) -> issue matmul one step late so the
        # ACT PSUM->SBUF copy overlaps the next transpose
        # first chunk kept small so e_0 (which gates the scan) is ready fast
        chunks = [8, 56] + [TC] * ((S - TC) // TC)
        assert sum(chunks) == S
        t0 = 0
        for clen in chunks:
            stage = spool.tile([BSH, TC * D], F32, tag="stage")
            nc.sync.dma_start(
                out=stage[:, : clen * D],
                in_=x[:, t0 : t0 + clen, :].rearrange("b t d -> b (t d)"),
            )
            for tl in range(clen):
                t = t0 + tl
                xt_ps = ppool_xt.tile([D, BSH], F32, tag="xt")
                nc.tensor.transpose(xt_ps[:], stage[:, tl * D : (tl + 1) * D], ident[:])
                xt_sb = wpool.tile([D, BSH], F32, tag="xts")
                nc.scalar.copy(out=xt_sb[:], in_=xt_ps[:])
                if pending is not None:
                    pt, psb = pending
                    e_ps = ppool.tile([BSH, K], F32, tag="e")
                    nc.tensor.matmul(e_ps[:], psb[:], wt[:], start=True, stop=True)
                    nc.scalar.copy(out=e_hist[:, pt * K : (pt + 1) * K], in_=e_ps[:])
                pending = (t, xt_sb)
            t0 += clen
        pt, psb = pending
        e_ps = ppool.tile([BSH, K], F32, tag="e")
        nc.tensor.matmul(e_ps[:], psb[:], wt[:], start=True, stop=True)
        nc.scalar.copy(out=e_hist[:, pt * K : (pt + 1) * K], in_=e_ps[:])

        # ---------------- forward scan (DVE) ----------------
        # view (j, k)-flat T as [p, k, j] (j innermost, stride K)
        ttbc3 = ttbc[:].rearrange("p (j k) -> p k j", k=K)
        nc.vector.tensor_copy(hist[:, 0:K], e_hist[:, 0:K])
        n_fwd = S if build_stage in ("full", "fwd") else 1
        for t in range(1, n_fwd):
            prev = (
                hist[:, (t - 1) * K : t * K]
                .rearrange("p (o j) -> p o j", o=1)
                .to_broadcast([BSH, K, K])
            )
            scores = wpool.tile([BSH, KK], F32, tag="scores")
            s3 = scores[:].rearrange("p (k j) -> p k j", j=K)
            nc.vector.tensor_add(s3, prev, ttbc3)
            m = wpool.tile([BSH, K], F32, tag="m")
            nc.vector.reduce_max(m[:], s3, axis=AX.X)
            nc.vector.tensor_add(
                hist[:, t * K : (t + 1) * K], m[:], e_hist[:, t * K : (t + 1) * K]
            )

        # ---------------- backtrack ----------------
        # y kept as fp32 during the walk (it feeds the one-hot compare); cast
        # to int32 once at the end.
        y_hist = hpool.tile([BSH, S], F32)
        y_hist_i = hpool.tile([BSH, S], mybir.dt.int32)

        # t = S-1: argmax over the final deltas
        tmp = hist[:, (S - 1) * K : S * K]
        max8 = btpool.tile([BSH, 8], F32, tag="max8")
        nc.vector.max(out=max8[:], in_=tmp)
        idx8 = btpool.tile([BSH, 8], mybir.dt.uint32, tag="idx8")
        nc.vector.max_index(idx8[:], max8[:], tmp)
        nc.vector.tensor_copy(y_hist[:, S - 1 : S], idx8[:, 0:1])

        bt_stop = -1 if build_stage == "full" else S - 3  # partial builds: one bt step
        for t in range(S - 2, bt_stop, -1):
            # one-hot of y_{t+1} straight from idx8 (uint32) so the y_hist
            # write stays off the critical chain
            onehot = btpool.tile([BSH, K], F32, tag="oh")
            nc.vector.tensor_tensor(
                onehot[:],
                iota_f[:],
                idx8[:, 0:1].to_broadcast([BSH, K]),
                op=OP.is_equal,
            )
            ohT_ps = ppool_bt.tile([K, BSH], F32, tag="bt")
            nc.tensor.transpose(ohT_ps[:], onehot[:], ident[:])
            ohT = btpool.tile([K, BSH], F32, tag="ohT")
            nc.vector.tensor_copy(ohT[:], ohT_ps[:])
            tcol_ps = ppool_bt.tile([BSH, K], F32, tag="bt")
            nc.tensor.matmul(tcol_ps[:], ohT[:], tt_T[:], start=True, stop=True)
            tmp2 = btpool.tile([BSH, K], F32, tag="tmp2")
            nc.vector.tensor_add(tmp2[:], hist[:, t * K : (t + 1) * K], tcol_ps[:])
            max8 = btpool.tile([BSH, 8], F32, tag="max8")
            nc.vector.max(out=max8[:], in_=tmp2[:])
            idx8 = btpool.tile([BSH, 8], mybir.dt.uint32, tag="idx8")
            nc.vector.max_index(idx8[:], max8[:], tmp2[:])
            nc.vector.tensor_copy(y_hist[:, t : t + 1], idx8[:, 0:1])

        nc.vector.tensor_copy(y_hist_i[:], y_hist[:])
        nc.sync.dma_start(out=y_out[:], in_=y_hist_i[:])

    n = _split_multiwaits(nc)
    if n:
        import logging

        logging.getLogger(__name__).info("split %d multi-wait instructions", n)
    return nc


def run(input_x, weights, transition, **spmd_kwargs):
    from concourse.bass_utils import run_bass_kernel_spmd

    nc = _build()
    input_x = np.ascontiguousarray(np.asarray(input_x, dtype=np.float32))
    weights = np.ascontiguousarray(np.asarray(weights, dtype=np.float32))
    transition = np.ascontiguousarray(np.asarray(transition, dtype=np.float32))
    in_maps = [
        {
            "x": input_x[i * BSH : (i + 1) * BSH],
            "w": weights,
            "t": transition,
        }
        for i in range(NCORES)
    ]
    res = run_bass_kernel_spmd(nc, in_maps, core_ids=list(range(NCORES)), **spmd_kwargs)
    out = np.concatenate([r["y"] for r in res.results], axis=0).astype(np.int32)
    return out, res


def kernel(input_x, weights, transition):
    out, _ = run(input_x, weights, transition)
    return out



# revision 4
# speedup vs baseline: 1.1110x; 1.1110x over previous
"""Batched Viterbi (max-sum) CRF decode on 8 Trainium2 NeuronCores.

Problem: input_x [1024, 256, 128] f32, weights [26, 128], transition [26, 26].
emissions e = x @ W^T; forward scan delta_t[k] = max_j(delta_{t-1}[j] + T[j,k]) + e_t[k];
backtrack the argmax path. Output: labels [1024, 256] int32.

Sharding: pure data parallel - batch 1024 split over 8 cores (128 rows/core, one
batch row per SBUF partition). Weights/transition replicated.

Forward scan: ONE tensor_tensor_scan per step over a 676-wide (k-outer,
j-inner) T table computes all 26 windowed max-plus reductions:
  state'_j = max(state'_{j-1} + ddiff_j, T[j,k]),  ddiff_j = delta_{j-1}-delta_j
with -BIG in ddiff slot 0 resetting each window; window ends hold
max_j(delta_j + T[j,k]) - delta_25. Global offsets cancel in every argmax, so
the recursion tracks pseudo-deltas pd_t = scanout_ends + e_t (no offset fixup).

Backtrack: per-step recompute prev = argmax_j(pd_t[j] + T[j,y]) via one-hot of
y transposed on PE and multiplied with T^T; batch is split in two halves whose
chains interleave, hiding the cross-engine latency. y kept as uint32 max_index
output slots; converted/gathered once at the end.

This container's walrus accepts at most one semaphore wait per instruction,
while Tile emits several on the kernel-tail drain - patched below by splitting
waits onto chained drains / NoOps. GPSIMD software ops don't codegen here.
"""

import functools

import numpy as np

B, S, D, K = 1024, 256, 128, 26
NCORES = 8
BSH = B // NCORES  # 128 batch rows per core == SBUF partition count
KK = K * K  # 676
TC = 64  # time steps per x-staging chunk
NEG = -1.0e30


def _patch_tile_drain():
    """Split the kernel-tail drain's sem waits across chained drain
    instructions (this walrus allows one wait per instruction)."""
    import concourse.mybir as mybir
    from concourse.tile import TileContext
    from concourse.vector_clock import ScopedClock

    if getattr(TileContext, "_drain_split_patched", False):
        return

    def patched(self, tick_clock, wait_clock):
        nc = self.nc
        drain_inst = nc.sync.drain()
        wait_clock.add_sem_waits(
            drain_inst.ins, ScopedClock({None: tick_clock.global_clock})
        )
        raw = drain_inst.ins
        si = raw.sync_info
        waits = list(si.on_wait)
        if len(waits) > 1:
            raw.sync_info = mybir.SyncInfo(
                on_wait=waits[:1], on_update=list(si.on_update)
            )
            for w in waits[1:]:
                extra = nc.sync.drain()
                extra.ins.sync_info = mybir.SyncInfo(on_wait=[w], on_update=[])
        nc.all_engine_barrier()
        popped = nc._tile_sem_poison_stack.pop()
        assert popped is self._sem_poison
        nc.clear_and_free_semaphores(list(self.sems.allocated().values()))
        nc.all_engine_barrier()

    TileContext._drain_and_barrier = patched
    TileContext._drain_split_patched = True


def _split_multiwaits(nc):
    """Hoist extra sem waits (>1 per instruction) onto preceding NoOps."""
    import concourse.mybir as mybir

    cnt = 0
    for f in nc.m.functions:
        for bb in f.blocks:
            insts = bb.instructions
            new_list = []
            changed = False
            for inst in insts:
                si = getattr(inst, "sync_info", None)
                waits = list(si.on_wait) if si is not None else []
                if len(waits) > 1:
                    for w in waits[:-1]:
                        nop = mybir.InstNoOp(name=f"mwsplit-{cnt}", ins=[], outs=[])
                        cnt += 1
                        nop.engine = inst.engine
                        nop.sync_info = mybir.SyncInfo(on_wait=[w], on_update=[])
                        new_list.append(nop)
                    inst.sync_info = mybir.SyncInfo(
                        on_wait=[waits[-1]], on_update=list(si.on_update)
                    )
                    changed = True
                new_list.append(inst)
            if changed:
                insts[:] = new_list
    return cnt


def _ttss(nc, out, data0, data1, initial, op0, op1):
    """tensor_tensor_scan accepting a multi-free-dim (broadcast) data0 view.

    Mirrors BassVectorEngine.tensor_tensor_scan minus the 2D-only assert: the
    scan runs in flat AP iteration order, which for our [p, k(bcast), j] view
    is exactly the window-repeated ddiff sequence (verified on HW).
    """
    import concourse.mybir as mybir

    eng = nc.vector
    return eng.add_instruction(
        mybir.InstTensorScalarPtr(
            name=nc.get_next_instruction_name(),
            is_tensor_tensor_scan=True,
            is_scalar_tensor_tensor=True,
            op0=op0,
            op1=op1,
            ins=[
                eng.lower_ap(data0),
                eng.lower_ap_or_imm(initial),
                eng.lower_ap(data1),
            ],
            outs=[eng.lower_ap(out)],
        )
    )


@functools.cache
def _build(build_stage="full"):
    import concourse.bass as bass
    import concourse.mybir as mybir
    from concourse.tile import TileContext

    _patch_tile_drain()

    F32 = mybir.dt.float32
    U32 = mybir.dt.uint32
    OP = mybir.AluOpType

    nc = bass.Bass()
    x = nc.dram_tensor("x", [BSH, S, D], F32, kind="ExternalInput")
    w = nc.dram_tensor("w", [K, D], F32, kind="ExternalInput")
    t_in = nc.dram_tensor("t", [K, K], F32, kind="ExternalInput")
    y_out = nc.dram_tensor("y", [BSH, S], mybir.dt.int32, kind="ExternalOutput")

    ident_c = nc.inline_tensor(np.eye(BSH, dtype=np.float32), name="identc")
    iota_c = nc.inline_tensor(
        np.tile(np.arange(K, dtype=np.float32), (BSH, 1)), name="iotac"
    )
    ones_c = nc.inline_tensor(np.ones((1, BSH), dtype=np.float32), name="onesc")

    with (
        TileContext(nc) as tc,
        tc.tile_pool(name="const", bufs=1) as cpool,
        tc.tile_pool(name="hist", bufs=1) as hpool,
        tc.tile_pool(name="stage", bufs=2) as spool,
        tc.tile_pool(name="work", bufs=3) as wpool,
        tc.tile_pool(name="scan", bufs=2) as scpool,
        tc.tile_pool(name="bt", bufs=4) as btpool,
        tc.tile_pool(name="psum_e", bufs=2, space="PSUM") as ppool,
        tc.tile_pool(name="psum_xt", bufs=2, space="PSUM") as ppool_xt,
        tc.tile_pool(name="psum_bt", bufs=2, space="PSUM") as ppool_bt,
    ):
        # ---------------- constants ----------------
        ident = cpool.tile([BSH, BSH], F32)
        nc.sync.dma_start(out=ident[:], in_=ident_c[:])
        iota_f = cpool.tile([BSH, K], F32)
        nc.sync.dma_start(out=iota_f[:], in_=iota_c[:])
        ones1 = cpool.tile([1, BSH], F32)
        nc.sync.dma_start(out=ones1[:], in_=ones_c[:])

        wt = cpool.tile([D, K], F32)  # W^T [d, k]
        nc.sync.dma_start(out=wt[:], in_=w[:].rearrange("k d -> d k"))

        # T in (k-outer, j-inner) flat order on one partition, replicated to
        # all partitions via PE ones-matmul: tord[b, 26k+j] = T[j, k]
        tt0 = cpool.tile([1, KK], F32)
        nc.sync.dma_start(
            out=tt0[:].rearrange("o (k j) -> o k j", k=K),
            in_=t_in[:].rearrange("j (o k) -> o k j", o=1),
        )
        tord = cpool.tile([BSH, KK], F32)
        half = KK // 2  # 338: fits one PSUM bank
        for h in range(2):
            rep_ps = ppool_xt.tile([BSH, half], F32, tag="xt")
            nc.tensor.matmul(
                rep_ps[:],
                ones1[:],
                tt0[:, h * half : (h + 1) * half],
                start=True,
                stop=True,
            )
            nc.vector.tensor_copy(tord[:, h * half : (h + 1) * half], rep_ps[:])

        # T^T [k, j] for the backtrack column-select matmul
        t_sb = cpool.tile([K, K], F32)
        nc.sync.dma_start(out=t_sb[:], in_=t_in[:])
        ttr_ps = ppool_xt.tile([K, K], F32, tag="xt")
        nc.tensor.transpose(ttr_ps[:], t_sb[:], ident[:K, :K])
        tt_T = cpool.tile([K, K], F32)
        nc.scalar.copy(out=tt_T[:], in_=ttr_ps[:])

        # pseudo-delta history [b, t*K + k]; emissions staged to SBUF by ACT
        hist = hpool.tile([BSH, S * K], F32)
        e_hist = hpool.tile([BSH, S * K], F32)
        # ddiff[., 0] = -BIG resets each scan window; slots 1..25 rewritten
        # every step with adjacent pseudo-delta differences
        ddiff = hpool.tile([BSH, K], F32)
        nc.vector.memset(ddiff[:, 0:1], NEG)

        # ---------------- emissions (PE) ----------------
        pending = None  # issue each e-matmul one step late so the ACT
        # PSUM->SBUF copy overlaps the next transpose
        chunks = [8, 56] + [TC] * ((S - TC) // TC)
        assert sum(chunks) == S
        t0 = 0
        for clen in chunks:
            stage = spool.tile([BSH, TC * D], F32, tag="stage")
            nc.sync.dma_start(
                out=stage[:, : clen * D],
                in_=x[:, t0 : t0 + clen, :].rearrange("b t d -> b (t d)"),
            )
            for tl in range(clen):
                t = t0 + tl
                xt_ps = ppool_xt.tile([D, BSH], F32, tag="xt")
                nc.tensor.transpose(xt_ps[:], stage[:, tl * D : (tl + 1) * D], ident[:])
                xt_sb = wpool.tile([D, BSH], F32, tag="xts")
                nc.scalar.copy(out=xt_sb[:], in_=xt_ps[:])
                if pending is not None:
                    pt, psb = pending
                    e_ps = ppool.tile([BSH, K], F32, tag="e")
                    nc.tensor.matmul(e_ps[:], psb[:], wt[:], start=True, stop=True)
                    nc.scalar.copy(out=e_hist[:, pt * K : (pt + 1) * K], in_=e_ps[:])
                pending = (t, xt_sb)
            t0 += clen
        pt, psb = pending
        e_ps = ppool.tile([BSH, K], F32, tag="e")
        nc.tensor.matmul(e_ps[:], psb[:], wt[:], start=True, stop=True)
        nc.scalar.copy(out=e_hist[:, pt * K : (pt + 1) * K], in_=e_ps[:])

        # ---------------- forward scan (DVE) ----------------
        # t = 0: pseudo-delta = e_0
        nc.vector.tensor_copy(hist[:, 0:K], e_hist[:, 0:K])
        nc.vector.tensor_tensor(
            out=ddiff[:, 1:K],
            in0=hist[:, 0 : K - 1],
            in1=hist[:, 1:K],
            op=OP.subtract,
        )
        n_fwd = S if build_stage in ("full", "fwd") else 2
        for t in range(1, n_fwd):
            scanout = scpool.tile([BSH, KK], F32, tag="scan")
            d0 = (
                ddiff[:]
                .rearrange("p (o j) -> p o j", o=1)
                .to_broadcast([BSH, K, K])
            )
            _ttss(nc, scanout[:], d0, tord[:], NEG, OP.add, OP.max)
            hs = hist[:, t * K : (t + 1) * K]
            nc.vector.tensor_tensor(
                out=hs,
                in0=scanout[:, K - 1 : KK : K],
                in1=e_hist[:, t * K : (t + 1) * K],
                op=OP.add,
            )
            if t < S - 1:
                nc.vector.tensor_tensor(
                    out=ddiff[:, 1:K],
                    in0=hist[:, t * K : (t + 1) * K - 1],
                    in1=hist[:, t * K + 1 : (t + 1) * K],
                    op=OP.subtract,
                )

        # ---------------- backtrack ----------------
        # y held as uint32 max_index slots [b, 8*t]; two batch halves (G=2)
        # interleave so the cross-engine chain latency is hidden.
        yh = hpool.tile([BSH, 8 * S], U32)
        H = BSH // 2
        halves = ((0, H), (H, BSH))

        # t = S-1: argmax over the final pseudo-deltas (full width, cheap)
        tmp = hist[:, (S - 1) * K : S * K]
        max8 = btpool.tile([BSH, 8], F32, tag="max8")
        nc.vector.max(out=max8[:], in_=tmp)
        nc.vector.max_index(yh[:, 8 * (S - 1) : 8 * S], max8[:], tmp)

        bt_stop = -1 if build_stage == "full" else S - 3
        for t in range(S - 2, bt_stop, -1):
            ohT = btpool.tile([K, BSH], F32, tag="ohT")
            tcol_ps = ppool_bt.tile([BSH, K], F32, tag="bt")
            tmp2 = btpool.tile([BSH, K], F32, tag="tmp2")
            max8 = btpool.tile([BSH, 8], F32, tag="max8")
            onehot = btpool.tile([BSH, K], F32, tag="oh")
            for lo, hi in halves:
                n = hi - lo
                nc.vector.tensor_tensor(
                    onehot[lo:hi, :],
                    iota_f[lo:hi, :],
                    yh[lo:hi, 8 * (t + 1) : 8 * (t + 1) + 1].to_broadcast([n, K]),
                    op=OP.is_equal,
                )
                ohT_ps = ppool_bt.tile([K, BSH], F32, tag="btx")
                nc.tensor.transpose(
                    ohT_ps[:, lo:hi], onehot[lo:hi, :], ident[lo:hi, lo:hi]
                )
                nc.scalar.copy(out=ohT[:, lo:hi], in_=ohT_ps[:, lo:hi])
                nc.tensor.matmul(
                    tcol_ps[lo:hi, :], ohT[:, lo:hi], tt_T[:], start=True, stop=True
                )
                nc.vector.tensor_tensor(
                    tmp2[lo:hi, :],
                    hist[lo:hi, t * K : (t + 1) * K],
                    tcol_ps[lo:hi, :],
                    op=OP.add,
                )
                nc.vector.max(out=max8[lo:hi, :], in_=tmp2[lo:hi, :])
                nc.vector.max_index(
                    yh[lo:hi, 8 * t : 8 * t + 8], max8[lo:hi, :], tmp2[lo:hi, :]
                )

        # gather uint32 slots -> int32 labels, then DMA out
        y_i = hpool.tile([BSH, S], mybir.dt.int32)
        nc.vector.tensor_copy(y_i[:], yh[:, 0 : 8 * S : 8])
        nc.sync.dma_start(out=y_out[:], in_=y_i[:])

    n = _split_multiwaits(nc)
    if n:
        import logging

        logging.getLogger(__name__).info("split %d multi-wait instructions", n)
    return nc


def run(input_x, weights, transition, **spmd_kwargs):
    from concourse.bass_utils import run_bass_kernel_spmd

    nc = _build()
    input_x = np.ascontiguousarray(np.asarray(input_x, dtype=np.float32))
    weights = np.ascontiguousarray(np.asarray(weights, dtype=np.float32))
    transition = np.ascontiguousarray(np.asarray(transition, dtype=np.float32))
    in_maps = [
        {
            "x": input_x[i * BSH : (i + 1) * BSH],
            "w": weights,
            "t": transition,
        }
        for i in range(NCORES)
    ]
    res = run_bass_kernel_spmd(nc, in_maps, core_ids=list(range(NCORES)), **spmd_kwargs)
    out = np.concatenate([r["y"] for r in res.results], axis=0).astype(np.int32)
    return out, res


def kernel(input_x, weights, transition):
    out, _ = run(input_x, weights, transition)
    return out


# revision 5
# speedup vs baseline: 1.2505x; 1.1255x over previous
"""Batched Viterbi (max-sum) CRF decode on 8 Trainium2 NeuronCores.

Problem: input_x [1024, 256, 128] f32, weights [26, 128], transition [26, 26].
emissions e = x @ W^T; forward scan delta_t[k] = max_j(delta_{t-1}[j] + T[j,k]) + e_t[k];
backtrack the argmax path. Output: labels [1024, 256] int32.

Sharding: pure data parallel - batch 1024 split over 8 cores (128 rows/core, one
batch row per SBUF partition). Weights/transition replicated.

Forward scan: ONE tensor_tensor_scan per step over a 676-wide (k-outer,
j-inner) T table computes all 26 windowed max-plus reductions:
  state'_j = max(state'_{j-1} + ddiff_j, T[j,k]),  ddiff_j = delta_{j-1}-delta_j
with -BIG in ddiff slot 0 resetting each window; window ends hold
max_j(delta_j + T[j,k]) - delta_25. Global offsets cancel in every argmax, so
the recursion tracks pseudo-deltas pd_t = scanout_ends + e_t (no offset fixup).

Backtrack: per-step recompute prev = argmax_j(pd_t[j] + T[j,y]) via one-hot of
y transposed on PE and multiplied with T^T; batch is split in two halves whose
chains interleave, hiding the cross-engine latency. y kept as uint32 max_index
output slots; converted/gathered once at the end.

This container's walrus accepts at most one semaphore wait per instruction,
while Tile emits several on the kernel-tail drain - patched below by splitting
waits onto chained drains / NoOps. GPSIMD software ops don't codegen here.
"""

import functools

import numpy as np

B, S, D, K = 1024, 256, 128, 26
NCORES = 8
BSH = B // NCORES  # 128 batch rows per core == SBUF partition count
KK = K * K  # 676
TC = 64  # time steps per x-staging chunk
NEG = -1.0e30


def _patch_tile_drain():
    """Split the kernel-tail drain's sem waits across chained drain
    instructions (this walrus allows one wait per instruction)."""
    import concourse.mybir as mybir
    from concourse.tile import TileContext
    from concourse.vector_clock import ScopedClock

    if getattr(TileContext, "_drain_split_patched", False):
        return

    def patched(self, tick_clock, wait_clock):
        nc = self.nc
        drain_inst = nc.sync.drain()
        wait_clock.add_sem_waits(
            drain_inst.ins, ScopedClock({None: tick_clock.global_clock})
        )
        raw = drain_inst.ins
        si = raw.sync_info
        waits = list(si.on_wait)
        if len(waits) > 1:
            raw.sync_info = mybir.SyncInfo(
                on_wait=waits[:1], on_update=list(si.on_update)
            )
            for w in waits[1:]:
                extra = nc.sync.drain()
                extra.ins.sync_info = mybir.SyncInfo(on_wait=[w], on_update=[])
        nc.all_engine_barrier()
        popped = nc._tile_sem_poison_stack.pop()
        assert popped is self._sem_poison
        nc.clear_and_free_semaphores(list(self.sems.allocated().values()))
        nc.all_engine_barrier()

    TileContext._drain_and_barrier = patched
    TileContext._drain_split_patched = True


def _split_multiwaits(nc):
    """Hoist extra sem waits (>1 per instruction) onto preceding NoOps."""
    import concourse.mybir as mybir

    cnt = 0
    for f in nc.m.functions:
        for bb in f.blocks:
            insts = bb.instructions
            new_list = []
            changed = False
            for inst in insts:
                si = getattr(inst, "sync_info", None)
                waits = list(si.on_wait) if si is not None else []
                if len(waits) > 1:
                    for w in waits[:-1]:
                        nop = mybir.InstNoOp(name=f"mwsplit-{cnt}", ins=[], outs=[])
                        cnt += 1
                        nop.engine = inst.engine
                        nop.sync_info = mybir.SyncInfo(on_wait=[w], on_update=[])
                        new_list.append(nop)
                    inst.sync_info = mybir.SyncInfo(
                        on_wait=[waits[-1]], on_update=list(si.on_update)
                    )
                    changed = True
                new_list.append(inst)
            if changed:
                insts[:] = new_list
    return cnt


def _ttss(nc, out, data0, data1, initial, op0, op1):
    """tensor_tensor_scan accepting a multi-free-dim (broadcast) data0 view.

    Mirrors BassVectorEngine.tensor_tensor_scan minus the 2D-only assert: the
    scan runs in flat AP iteration order, which for our [p, k(bcast), j] view
    is exactly the window-repeated ddiff sequence (verified on HW).
    """
    import concourse.mybir as mybir

    eng = nc.vector
    return eng.add_instruction(
        mybir.InstTensorScalarPtr(
            name=nc.get_next_instruction_name(),
            is_tensor_tensor_scan=True,
            is_scalar_tensor_tensor=True,
            op0=op0,
            op1=op1,
            ins=[
                eng.lower_ap(data0),
                eng.lower_ap_or_imm(initial),
                eng.lower_ap(data1),
            ],
            outs=[eng.lower_ap(out)],
        )
    )


@functools.cache
def _build(build_stage="full"):
    import concourse.bass as bass
    import concourse.mybir as mybir
    from concourse.tile import TileContext

    _patch_tile_drain()

    F32 = mybir.dt.float32
    U32 = mybir.dt.uint32
    OP = mybir.AluOpType

    nc = bass.Bass()
    x = nc.dram_tensor("x", [BSH, S, D], F32, kind="ExternalInput")
    w = nc.dram_tensor("w", [K, D], F32, kind="ExternalInput")
    t_in = nc.dram_tensor("t", [K, K], F32, kind="ExternalInput")
    y_out = nc.dram_tensor("y", [BSH, S], mybir.dt.int32, kind="ExternalOutput")

    ident_c = nc.inline_tensor(np.eye(BSH, dtype=np.float32), name="identc")
    iota_c = nc.inline_tensor(
        np.tile(np.arange(K, dtype=np.float32), (BSH, 1)), name="iotac"
    )
    ones_c = nc.inline_tensor(np.ones((1, BSH), dtype=np.float32), name="onesc")

    with (
        TileContext(nc) as tc,
        tc.tile_pool(name="const", bufs=1) as cpool,
        tc.tile_pool(name="hist", bufs=1) as hpool,
        tc.tile_pool(name="stage", bufs=2) as spool,
        tc.tile_pool(name="work", bufs=3) as wpool,
        tc.tile_pool(name="scan", bufs=2) as scpool,
        tc.tile_pool(name="bt", bufs=4) as btpool,
        tc.tile_pool(name="psum_e", bufs=2, space="PSUM") as ppool,
        tc.tile_pool(name="psum_xt", bufs=2, space="PSUM") as ppool_xt,
        tc.tile_pool(name="psum_bt", bufs=2, space="PSUM") as ppool_bt,
    ):
        # ---------------- constants ----------------
        ident = cpool.tile([BSH, BSH], F32)
        nc.sync.dma_start(out=ident[:], in_=ident_c[:])
        iota_f = cpool.tile([BSH, K], F32)
        nc.sync.dma_start(out=iota_f[:], in_=iota_c[:])
        ones1 = cpool.tile([1, BSH], F32)
        nc.sync.dma_start(out=ones1[:], in_=ones_c[:])

        wt = cpool.tile([D, K], F32)  # W^T [d, k]
        nc.sync.dma_start(out=wt[:], in_=w[:].rearrange("k d -> d k"))

        # T in (k-outer, j-inner) flat order on one partition, replicated to
        # all partitions via PE ones-matmul: tord[b, 26k+j] = T[j, k]
        tt0 = cpool.tile([1, KK], F32)
        nc.sync.dma_start(
            out=tt0[:].rearrange("o (k j) -> o k j", k=K),
            in_=t_in[:].rearrange("j (o k) -> o k j", o=1),
        )
        tord = cpool.tile([BSH, KK], F32)
        half = KK // 2  # 338: fits one PSUM bank
        for h in range(2):
            rep_ps = ppool_xt.tile([BSH, half], F32, tag="xt")
            nc.tensor.matmul(
                rep_ps[:],
                ones1[:],
                tt0[:, h * half : (h + 1) * half],
                start=True,
                stop=True,
            )
            nc.vector.tensor_copy(tord[:, h * half : (h + 1) * half], rep_ps[:])

        # T^T [k, j] for the backtrack column-select matmul
        t_sb = cpool.tile([K, K], F32)
        nc.sync.dma_start(out=t_sb[:], in_=t_in[:])
        ttr_ps = ppool_xt.tile([K, K], F32, tag="xt")
        nc.tensor.transpose(ttr_ps[:], t_sb[:], ident[:K, :K])
        tt_T = cpool.tile([K, K], F32)
        nc.scalar.copy(out=tt_T[:], in_=ttr_ps[:])

        # pseudo-delta history [b, t*K + k]; emissions staged to SBUF by ACT
        hist = hpool.tile([BSH, S * K], F32)
        e_hist = hpool.tile([BSH, S * K], F32)
        # ddiff[., 0] = -BIG resets each scan window; slots 1..25 rewritten
        # every step with adjacent pseudo-delta differences
        ddiff = hpool.tile([BSH, K], F32)
        nc.vector.memset(ddiff[:, 0:1], NEG)

        # ---------------- emissions (PE) ----------------
        pending = None  # issue each e-matmul one step late so the ACT
        # PSUM->SBUF copy overlaps the next transpose
        chunks = [8, 56] + [TC] * ((S - TC) // TC)
        assert sum(chunks) == S
        t0 = 0
        for clen in chunks:
            stage = spool.tile([BSH, TC * D], F32, tag="stage")
            nc.sync.dma_start(
                out=stage[:, : clen * D],
                in_=x[:, t0 : t0 + clen, :].rearrange("b t d -> b (t d)"),
            )
            for tl in range(clen):
                t = t0 + tl
                xt_ps = ppool_xt.tile([D, BSH], F32, tag="xt")
                nc.tensor.transpose(xt_ps[:], stage[:, tl * D : (tl + 1) * D], ident[:])
                xt_sb = wpool.tile([D, BSH], F32, tag="xts")
                nc.scalar.copy(out=xt_sb[:], in_=xt_ps[:])
                if pending is not None:
                    pt, psb = pending
                    e_ps = ppool.tile([BSH, K], F32, tag="e")
                    nc.tensor.matmul(e_ps[:], psb[:], wt[:], start=True, stop=True)
                    nc.scalar.copy(out=e_hist[:, pt * K : (pt + 1) * K], in_=e_ps[:])
                pending = (t, xt_sb)
            t0 += clen
        pt, psb = pending
        e_ps = ppool.tile([BSH, K], F32, tag="e")
        nc.tensor.matmul(e_ps[:], psb[:], wt[:], start=True, stop=True)
        nc.scalar.copy(out=e_hist[:, pt * K : (pt + 1) * K], in_=e_ps[:])

        # ---------------- forward scan (DVE) ----------------
        # t = 0: pseudo-delta = e_0
        nc.vector.tensor_copy(hist[:, 0:K], e_hist[:, 0:K])
        nc.vector.tensor_tensor(
            out=ddiff[:, 1:K],
            in0=hist[:, 0 : K - 1],
            in1=hist[:, 1:K],
            op=OP.subtract,
        )
        n_fwd = S if build_stage in ("full", "fwd") else 2
        for t in range(1, n_fwd):
            scanout = scpool.tile([BSH, KK], F32, tag="scan")
            d0 = (
                ddiff[:]
                .rearrange("p (o j) -> p o j", o=1)
                .to_broadcast([BSH, K, K])
            )
            _ttss(nc, scanout[:], d0, tord[:], NEG, OP.add, OP.max)
            hs = hist[:, t * K : (t + 1) * K]
            nc.vector.tensor_tensor(
                out=hs,
                in0=scanout[:, K - 1 : KK : K],
                in1=e_hist[:, t * K : (t + 1) * K],
                op=OP.add,
            )
            if t < S - 1:
                nc.vector.tensor_tensor(
                    out=ddiff[:, 1:K],
                    in0=hist[:, t * K : (t + 1) * K - 1],
                    in1=hist[:, t * K + 1 : (t + 1) * K],
                    op=OP.subtract,
                )

        # ---------------- backtrack ----------------
        # Chain state per batch half is the ONE-HOT of y (not the index):
        #   onehot -> PE transpose -> ACT copy -> PE mm1 (T-column select) +
        #   PE mm2 (identity-matmul accumulates hist into the same PSUM) ->
        #   DVE max8 -> DVE is_equal (next onehot straight from the max value).
        # max_index (the label itself) runs off the critical chain into uint32
        # slot history. Two batch halves interleave to hide chain latency.
        yh = hpool.tile([BSH, 8 * S], U32)
        H = BSH // 2
        halves = ((0, H), (H, BSH))

        # t = S-1: argmax over the final pseudo-deltas
        oh_prev = {}
        for lo, hi in halves:
            n = hi - lo
            tmp = hist[lo:hi, (S - 1) * K : S * K]
            max8 = btpool.tile([BSH, 8], F32, tag="max8")
            nc.vector.max(out=max8[lo:hi, :], in_=tmp)
            onehot = btpool.tile([BSH, K], F32, tag="oh")
            nc.vector.tensor_tensor(
                onehot[lo:hi, :],
                tmp,
                max8[lo:hi, 0:1].to_broadcast([n, K]),
                op=OP.is_equal,
            )
            nc.vector.max_index(yh[lo:hi, 8 * (S - 1) : 8 * S], max8[lo:hi, :], tmp)
            oh_prev[lo] = onehot

        bt_stop = -1 if build_stage == "full" else S - 3
        for t in range(S - 2, bt_stop, -1):
            for lo, hi in halves:
                n = hi - lo
                onehot = oh_prev[lo]
                ohT_ps = ppool_bt.tile([K, BSH], F32, tag="btx")
                nc.tensor.transpose(
                    ohT_ps[:, lo:hi], onehot[lo:hi, :], ident[lo:hi, lo:hi]
                )
                ohT = btpool.tile([K, BSH], F32, tag="ohT")
                nc.scalar.copy(out=ohT[:, lo:hi], in_=ohT_ps[:, lo:hi])
                tcol_ps = ppool_bt.tile([BSH, K], F32, tag="bt")
                nc.tensor.matmul(
                    tcol_ps[lo:hi, :], ohT[:, lo:hi], tt_T[:], start=True, stop=False
                )
                nc.tensor.matmul(
                    tcol_ps[lo:hi, :],
                    ident[lo:hi, lo:hi],
                    hist[lo:hi, t * K : (t + 1) * K],
                    start=False,
                    stop=True,
                )
                max8 = btpool.tile([BSH, 8], F32, tag="max8")
                nc.vector.max(out=max8[lo:hi, :], in_=tcol_ps[lo:hi, :])
                oh_next = btpool.tile([BSH, K], F32, tag="oh")
                nc.vector.tensor_tensor(
                    oh_next[lo:hi, :],
                    tcol_ps[lo:hi, :],
                    max8[lo:hi, 0:1].to_broadcast([n, K]),
                    op=OP.is_equal,
                )
                nc.vector.max_index(
                    yh[lo:hi, 8 * t : 8 * t + 8],
                    max8[lo:hi, :],
                    tcol_ps[lo:hi, :],
                )
                oh_prev[lo] = oh_next

        # gather uint32 slots -> int32 labels, then DMA out
        y_i = hpool.tile([BSH, S], mybir.dt.int32)
        nc.vector.tensor_copy(y_i[:], yh[:, 0 : 8 * S : 8])
        nc.sync.dma_start(out=y_out[:], in_=y_i[:])

    n = _split_multiwaits(nc)
    if n:
        import logging

        logging.getLogger(__name__).info("split %d multi-wait instructions", n)
    return nc


def run(input_x, weights, transition, **spmd_kwargs):
    from concourse.bass_utils import run_bass_kernel_spmd

    nc = _build()
    input_x = np.ascontiguousarray(np.asarray(input_x, dtype=np.float32))
    weights = np.ascontiguousarray(np.asarray(weights, dtype=np.float32))
    transition = np.ascontiguousarray(np.asarray(transition, dtype=np.float32))
    in_maps = [
        {
            "x": input_x[i * BSH : (i + 1) * BSH],
            "w": weights,
            "t": transition,
        }
        for i in range(NCORES)
    ]
    res = run_bass_kernel_spmd(nc, in_maps, core_ids=list(range(NCORES)), **spmd_kwargs)
    out = np.concatenate([r["y"] for r in res.results], axis=0).astype(np.int32)
    return out, res


def kernel(input_x, weights, transition):
    out, _ = run(input_x, weights, transition)
    return out
